# revision 7
# baseline (speedup 1.0000x reference)
"""Trainium2 Bass kernel: ResNet BasicBlock (conv3x3-BN-ReLU-mask-conv3x3-mask-BN-residual-ReLU).

Problem shape: x[4096, 64, 7, 7], both convs 64->64 3x3 pad 1.

Strategy (pure data parallel, 8 cores, 512 images/core):
  * Channels live on SBUF partitions. Two 64-channel image streams are
    stacked into the 128 partitions ("lane0" -> partitions 0-63,
    "lane1" -> 64-127) so elementwise engines run at full width.
  * A 3x3 conv is 9 shifted 64x64 matmuls accumulated in PSUM. Images are
    zero-padded to 9x9 on-chip; each tap reads a strided window of the
    padded tile. Matmul operands are bf16 (fp32 matmuls lower to multiple
    PE passes); accumulation stays fp32 in PSUM.
  * The 128x128 PE array is split into 4 64x64 quadrants via the matmul
    base partitions. Four independent chains (2 pairs x 2 lanes) run
    concurrently, fully utilizing the array despite C=64.
  * Tap-OUTER loop over superblocks of 4 rounds (4 PSUM-bank-pairs): all
    rounds of a tap run back-to-back with the same stationary weights, and
    redundant LDWEIGHTS are deleted from the BIR post-schedule (the PE
    reuses loaded weights), eliminating the per-matmul weight-load tax.
  * BN scales fold into conv weights on the host; BN shifts are
    per-partition bias operands. The identity residual is added in fp32 by
    VectorE directly into PSUM before the final relu.
  * x and the output travel as bf16 over DMA (residual picks up <=2^-9
    relative rounding - well within the error budget), halving HBM traffic.
  * Critic masks only touch batch element 0: every core runs the same mask
    multiply on its first image; cores 1-7 get all-ones masks.
"""

import ml_dtypes
import numpy as np

import concourse.bass as bass  # noqa: F401  (engine namespaces live on the nc object)
import concourse.tile as tile
from concourse import bacc, mybir
from concourse.bass_utils import run_bass_kernel_spmd

F32 = mybir.dt.float32
BF16 = mybir.dt.bfloat16
NP_BF16 = ml_dtypes.bfloat16
EPS = 1e-5
B, C, H, W = 4096, 64, 7, 7
NCORES = 8
BPC = B // NCORES          # 512 images per core
SLOTS = BPC // 2           # 256 image slots per lane
N = 8                      # images per chain visit
NR = 16                    # rounds (each = 4 chains x 8 images = 32 images)
NSB = 4                    # superblocks of 4 rounds
RPS = 4                    # rounds per superblock
FD = N * H * W             # 392 psum columns per chain

# (pair_in_round, lane, colgroup): the 4 concurrent chains of a round.
# Even pair writes PSUM naturally, odd pair swapped - this keeps all four
# PE quadrants busy. The odd pair's lanes swap in y1pad and swap back
# after conv2, so the final output is lane-aligned for the residual.
CHAINS = [(0, 0, 0), (1, 1, 0), (0, 1, 1), (1, 0, 1)]

_CACHE = {}


def _psum_view(ps, j, n=N):
    """[128, n, 7, 7] view of pair j's bank of a [128, 2, 512] psum tile."""
    return ps[:, j, 0:n * H * W].rearrange(
        "p (i h w) -> p i h w", i=n, h=H, w=W)


def _dedup_ldweights(nc):
    """Remove InstLdweights whose matmul reuses the weights already loaded
    into the same PE quadrant (tile_position). The PE array retains its
    stationary operand between matmuls, so a matmul with no preceding
    LDWEIGHTS streams against the previously loaded weights (verified on
    hardware). Sync waits/updates on a removed LDWEIGHTS migrate to its
    matmul."""
    kept = removed = 0
    for f in nc.m.functions:
        for b in f.blocks:
            insts = list(b.instructions)
            last = {}
            dead = []
            i = 0
            while i < len(insts):
                ins = insts[i]
                if isinstance(ins, mybir.InstLdweights):
                    assert i + 1 < len(insts), "trailing LDWEIGHTS"
                    mm = insts[i + 1]
                    assert isinstance(mm, mybir.InstMatmult), (
                        f"LDWEIGHTS not followed by matmul: {type(mm).__name__}")
                    sig = str(mm.ins[1])
                    tp = tuple(mm.tile_position)
                    if last.get(tp) == sig:
                        si = ins.sync_info
                        if si is not None and (len(si.on_wait) or len(si.on_update)):
                            msi = mm.sync_info
                            ow = list(si.on_wait)
                            ou = list(si.on_update)
                            if msi is not None:
                                ow += list(msi.on_wait)
                                ou += list(msi.on_update)
                            mm.sync_info = mybir.SyncInfo(on_wait=ow, on_update=ou)
                        dead.append(ins)
                        removed += 1
                    else:
                        last[tp] = sig
                        kept += 1
                    i += 2
                    continue
                assert not isinstance(ins, mybir.InstMatmult), "matmul without LDWEIGHTS"
                i += 1
            for d in dead:
                b.instructions.remove(d)
    # ideal tap-outer order keeps 324; the Tile scheduler hoists a few
    # next-tap matmuls into phase-transition gaps, costing extra keeps
    assert kept + removed == 1152 and removed >= 600, (kept, removed)


def _build():
    nc = bacc.Bacc("TRN2", target_bir_lowering=False, debug=False,
                   num_devices=NCORES)
    x_d = nc.dram_tensor("x", [128, SLOTS, H, W], BF16, kind="ExternalInput")
    w1_d = nc.dram_tensor("w1", [128, 9, 64], BF16, kind="ExternalInput")
    w2_d = nc.dram_tensor("w2", [128, 9, 64], BF16, kind="ExternalInput")
    cst_d = nc.dram_tensor("cst", [128, 2], F32, kind="ExternalInput")
    msk_d = nc.dram_tensor("msk", [64, 2, H, W], F32, kind="ExternalInput")
    o_d = nc.dram_tensor("o", [128, SLOTS, H, W], BF16, kind="ExternalOutput")

    with tile.TileContext(nc) as tc:
        with (
            tc.tile_pool(name="singles", bufs=1) as singles,
            tc.tile_pool(name="xin", bufs=8) as xin_pool,
            tc.tile_pool(name="outp", bufs=4) as out_pool,
            tc.tile_pool(name="pads", bufs=1) as pad_pool,
            tc.tile_pool(name="ps", bufs=4, space="PSUM") as ps_pool,
        ):
            w1_sb = singles.tile([128, 9, 64], BF16, name="w1_sb")
            w2_sb = singles.tile([128, 9, 64], BF16, name="w2_sb")
            cst_sb = singles.tile([128, 2], F32, name="cst_sb")
            msk_sb = singles.tile([64, 2, H, W], F32, name="msk_sb")
            warm_sb = singles.tile([128, 1], F32, name="warm_sb")

            # Persistent zero-padded 9x9 tiles: borders are zeroed once in
            # the prologue and never rewritten (compute writes interiors).
            xpads = [pad_pool.tile([128, 16, 9, 9], BF16, name=f"xpad{i}",
                                   tag=f"xpad{i}") for i in range(RPS)]
            y1pads = [pad_pool.tile([128, 16, 9, 9], BF16, name=f"y1pad{i}",
                                    tag=f"y1pad{i}") for i in range(RPS)]

            xin_tiles = {}

            def emit_in_dma(r, q):
                t = xin_pool.tile([128, 16, H, W], BF16, name="xin_q")
                q.dma_start(t[:], x_d[:, 16 * r:16 * r + 16])
                xin_tiles[r] = t

            def emit_pad_cast(r):
                nc.vector.tensor_copy(xpads[r % RPS][:, :, 1:8, 1:8],
                                      xin_tiles[r][:])

            def emit_chain_mms(pads, w_sb, ps_r, t, r):
                dh, dw = t // 3, t % 3
                for (j, lane, cg) in CHAINS:
                    rhs = pads[r % RPS][64 * lane:64 * lane + 64,
                                        8 * j:8 * j + 8, dh:dh + 7, dw:dw + 7]
                    lhsT = w_sb[64 * lane:64 * lane + 64, t, :]
                    out = ps_r[64 * cg:64 * cg + 64, j, 0:FD]
                    nc.tensor.matmul(out, lhsT, rhs, start=(t == 0),
                                     stop=(t == 8))

            def emit_drain(k, r, ps_r):
                # conv1 bank -> relu(psum + shift1) -> y1pad interior
                yp = y1pads[r % RPS]
                for j in range(2):
                    nc.scalar.activation(
                        out=yp[:, 8 * j:8 * j + 8, 1:8, 1:8],
                        in_=_psum_view(ps_r, j),
                        func=mybir.ActivationFunctionType.Relu,
                        bias=cst_sb[:, 0:1], scale=1.0)
                if k == 0 and r == 0:
                    # critic mask 1 on relu(bn1(conv1)) of batch element 0
                    tgt = yp[0:64, 0, 1:8, 1:8]
                    nc.vector.tensor_mul(tgt, tgt, msk_sb[:, 0, :, :])

            def emit_finish(k, r, ps_r):
                # conv2 bank: mask2 (elem 0) -> +residual -> relu -> DMA out
                if k == 0 and r == 0:
                    tgt = ps_r[0:64, 0, 0:H * W].rearrange(
                        "p (h w) -> p h w", h=H, w=W)
                    nc.vector.tensor_mul(tgt, tgt, msk_sb[:, 1, :, :])
                views = [_psum_view(ps_r, j) for j in range(2)]
                xin_q = xin_tiles[r]
                out_q = out_pool.tile([128, 16, H, W], BF16, name="out_q")
                for j in range(2):
                    nc.vector.tensor_add(
                        views[j], views[j], xin_q[:, 8 * j:8 * j + 8])
                nc.vector.tensor_scalar(
                    out_q[:, 0:8], views[0], cst_sb[:, 1:2], 0.0,
                    mybir.AluOpType.add, mybir.AluOpType.max)
                nc.scalar.activation(
                    out=out_q[:, 8:16], in_=views[1],
                    func=mybir.ActivationFunctionType.Relu,
                    bias=cst_sb[:, 1:2], scale=1.0)
                if r == NR - 1:
                    # final round: two half-DMAs so the kernel tail is short
                    nc.sync.dma_start(o_d[:, 16 * r:16 * r + 8], out_q[:, 0:8])
                    nc.sync.dma_start(o_d[:, 16 * r + 8:16 * r + 16],
                                      out_q[:, 8:16])
                else:
                    nc.sync.dma_start(o_d[:, 16 * r:16 * r + 16], out_q[:])

            # ---------------- prologue ----------------
            # DVE: first pad border zero (gates the first conv round)
            nc.vector.memset(xpads[0][:], 0.0)
            # ACT queue: w1 first (needed by the first LDWEIGHTS), then the
            # activation-table prewarm rides behind it
            nc.scalar.dma_start(w1_sb[:], w1_d[:])
            nc.scalar.memzero(warm_sb[:])
            # sync ring: first superblock's x
            for r in range(4):
                emit_in_dma(r, nc.sync)
            # constants on the SWDGE ring
            nc.gpsimd.dma_start(cst_sb[:], cst_d[:])
            nc.gpsimd.dma_start(w2_sb[:], w2_d[:])
            nc.gpsimd.dma_start(msk_sb[:], msk_d[:])
            # second superblock's x on the ACT ring
            for r in range(4, 8):
                emit_in_dma(r, nc.scalar)
            # remaining pad borders: DVE interleaved with the first casts;
            # xpad3 + y1pads off the critical path on gpsimd/scalar
            emit_pad_cast(0)
            nc.vector.memset(y1pads[0][:], 0.0)
            nc.vector.memset(xpads[1][:], 0.0)
            emit_pad_cast(1)
            nc.vector.memset(xpads[2][:], 0.0)
            emit_pad_cast(2)
            nc.gpsimd.memset(xpads[3][:], 0.0)
            emit_pad_cast(3)
            nc.gpsimd.memset(y1pads[1][:], 0.0)
            nc.gpsimd.memset(y1pads[2][:], 0.0)
            nc.gpsimd.memset(y1pads[3][:], 0.0)

            # ---------------- main loop ----------------
            for k in range(NSB):
                rounds = list(range(RPS * k, RPS * k + RPS))
                # conv1 phase (tap-outer over the superblock's rounds)
                ps1 = {r: ps_pool.tile([128, 2, 512], F32, name="ps_t")
                       for r in rounds}
                if k == 0:
                    # round 0 tap-inner first: it only needs the first DMA,
                    # so matmuls start while rounds 1-3 are still in flight
                    for t in range(9):
                        emit_chain_mms(xpads, w1_sb, ps1[0], t, 0)
                    emit_drain(k, 0, ps1[0])
                    for t in range(9):
                        for r in rounds[1:]:
                            emit_chain_mms(xpads, w1_sb, ps1[r], t, r)
                            if t == 8:
                                emit_drain(k, r, ps1[r])
                else:
                    for t in range(9):
                        for r in rounds:
                            emit_chain_mms(xpads, w1_sb, ps1[r], t, r)
                            if t == 8:
                                emit_drain(k, r, ps1[r])

                # prefetch: x DMAs for superblock k+2, pad casts for k+1
                if k + 2 < NSB:
                    for r in range(RPS * (k + 2), RPS * (k + 2) + RPS):
                        emit_in_dma(r, nc.sync)
                if k + 1 < NSB:
                    for r in range(RPS * (k + 1), RPS * (k + 1) + RPS):
                        emit_pad_cast(r)

                # conv2 phase
                ps2 = {r: ps_pool.tile([128, 2, 512], F32, name="ps_t")
                       for r in rounds}
                for t in range(9):
                    for r in rounds:
                        emit_chain_mms(y1pads, w2_sb, ps2[r], t, r)
                        if t == 8:
                            emit_finish(k, r, ps2[r])

    _dedup_ldweights(nc)
    nc.compile()
    return nc


def _get_nc():
    if "nc" not in _CACHE:
        _CACHE["nc"] = _build()
    return _CACHE["nc"]


# slot mapping: slot s (0..255) within a lane h decomposes as
#   k = s//64 (superblock), rr = (s//16)%4 (round), j = (s//8)%2 (pair),
#   i = s%8  ->  image index = 128k + 32rr + 16j + 8h + i
_S = np.arange(SLOTS)
_IMG = (128 * (_S // 64) + 32 * ((_S // 16) % 4) + 16 * ((_S // 8) % 2)
        + (_S % 8))
_IMG = np.stack([_IMG, _IMG + 8])          # [2 lanes, 256 slots]
_I = np.arange(BPC)
_INV_H = (_I % 16) // 8
_INV_S = 64 * (_I // 128) + 16 * ((_I % 128) // 32) + 8 * ((_I % 32) // 16) \
    + (_I % 8)


def _host_pack(x, w1, g1, b1, m1, v1, w2, g2, b2, m2, v2, mask1, mask2):
    x = np.asarray(x, np.float32)
    scale1 = np.asarray(g1, np.float32) / np.sqrt(np.asarray(v1, np.float32) + EPS)
    shift1 = np.asarray(b1, np.float32) - np.asarray(m1, np.float32) * scale1
    scale2 = np.asarray(g2, np.float32) / np.sqrt(np.asarray(v2, np.float32) + EPS)
    shift2 = np.asarray(b2, np.float32) - np.asarray(m2, np.float32) * scale2

    def pack_w(w, scale):
        ws = np.asarray(w, np.float32) * scale[:, None, None, None]
        # [co, ci, kh, kw] -> [ci, tap, co], duplicated into both lanes
        lhsT = ws.transpose(1, 2, 3, 0).reshape(64, 9, 64)
        return np.ascontiguousarray(np.tile(lhsT, (2, 1, 1)).astype(NP_BF16))

    wdev1, wdev2 = pack_w(w1, scale1), pack_w(w2, scale2)
    cst = np.tile(np.stack([shift1, shift2], 1), (2, 1))
    cst = np.ascontiguousarray(cst.astype(np.float32))

    x_cores = x.reshape(NCORES, BPC, C, H, W)
    # [core, lane, slot, ch, h, w] -> [core, lane*ch, slot, h, w]
    xdev = np.ascontiguousarray(
        x_cores[:, _IMG].transpose(0, 1, 3, 2, 4, 5).reshape(
            NCORES, 128, SLOTS, H, W).astype(NP_BF16))

    msk0 = np.ascontiguousarray(
        np.stack([np.asarray(mask1, np.float32),
                  np.asarray(mask2, np.float32)], 1))
    msk1s = np.ones_like(msk0)

    in_maps = []
    for c in range(NCORES):
        in_maps.append({
            "x": xdev[c],
            "w1": wdev1,
            "w2": wdev2,
            "cst": cst,
            "msk": msk0 if c == 0 else msk1s,
        })
    return in_maps


def _host_unpack(results):
    o = np.stack([results[c]["o"] for c in range(NCORES)])
    o = np.asarray(o, np.float32).reshape(NCORES, 2, C, SLOTS, H, W)
    o = o.transpose(0, 1, 3, 2, 4, 5)      # [core, lane, slot, ch, h, w]
    out = o[:, _INV_H, _INV_S]             # [core, img, ch, h, w]
    return np.ascontiguousarray(out.reshape(B, C, H, W))


def run(trace=False, **inputs):
    nc = _get_nc()
    in_maps = _host_pack(**inputs)
    res = run_bass_kernel_spmd(nc, in_maps, core_ids=list(range(NCORES)),
                               trace=trace)
    return _host_unpack(res.results), res


def kernel(**inputs) -> np.ndarray:
    out, _ = run(trace=False, **inputs)
    return out


# revision 8
# speedup vs baseline: 1.0669x; 1.0669x over previous
"""Trainium2 Bass kernel: ResNet BasicBlock (conv3x3-BN-ReLU-mask-conv3x3-mask-BN-residual-ReLU).

Problem shape: x[4096, 64, 7, 7], both convs 64->64 3x3 pad 1.

Strategy (pure data parallel, 8 cores, 512 images/core):
  * Channels live on SBUF partitions. Two 64-channel image streams are
    stacked into the 128 partitions ("lane0" -> partitions 0-63,
    "lane1" -> 64-127) so elementwise engines run at full width.
  * A 3x3 conv is 9 shifted 64x64 matmuls accumulated in PSUM. Images are
    zero-padded to 9x9 on-chip; each tap reads a strided window of the
    padded tile. Matmul operands are bf16; accumulation is fp32 in PSUM.
  * The 128x128 PE array is split into 4 64x64 quadrants via the matmul
    base partitions. Four independent chains (2 pairs x 2 lanes) run
    concurrently, fully utilizing the array despite C=64.
  * Tap-OUTER loop over superblocks of 4 rounds (4 PSUM-bank-pairs): all
    rounds of a tap run back-to-back with the same stationary weights, and
    redundant LDWEIGHTS are deleted from the BIR post-schedule (the PE
    reuses loaded weights), eliminating the per-matmul weight-load tax.
  * The identity residual is a full-array identity matmul that seeds each
    conv2 PSUM accumulation group with x (bf16 identity is exact), so no
    separate vector add sits on the bank-release critical path.
  * BN scales fold into conv weights on the host; BN shifts are
    per-partition bias operands of the drain/final activations.
  * The output travels back as bf16 (x stays fp32 inbound; the residual
    uses the bf16 conv input, costing <=2^-9 relative - within budget).
  * Critic masks only touch batch element 0: every core runs the same mask
    ops on its first image; cores 1-7 get neutral masks. Since the
    residual is pre-added, mask2 is applied as
    mask*(conv2+x) + (1-mask)*x on the first image's region.
"""

import ml_dtypes
import numpy as np

import concourse.bass as bass  # noqa: F401  (engine namespaces live on the nc object)
import concourse.tile as tile
from concourse import bacc, mybir
from concourse.bass_utils import run_bass_kernel_spmd

F32 = mybir.dt.float32
BF16 = mybir.dt.bfloat16
NP_BF16 = ml_dtypes.bfloat16
EPS = 1e-5
B, C, H, W = 4096, 64, 7, 7
NCORES = 8
BPC = B // NCORES          # 512 images per core
SLOTS = BPC // 2           # 256 image slots per lane
N = 8                      # images per chain visit
NR = 16                    # rounds (each = 4 chains x 8 images = 32 images)
NSB = 4                    # superblocks of 4 rounds
RPS = 4                    # rounds per superblock
NXP = 8                    # xpad ring depth (2 superblocks)
FD = N * H * W             # 392 psum columns per chain

# (pair_in_round, lane, colgroup): the 4 concurrent chains of a round.
# Even pair writes PSUM naturally, odd pair swapped - this keeps all four
# PE quadrants busy. The odd pair's lanes swap in y1pad and swap back
# after conv2, so the final output is lane-aligned.
CHAINS = [(0, 0, 0), (1, 1, 0), (0, 1, 1), (1, 0, 1)]

_CACHE = {}


def _psum_view(ps, j, n=N):
    """[128, n, 7, 7] view of pair j's bank of a [128, 2, 512] psum tile."""
    return ps[:, j, 0:n * H * W].rearrange(
        "p (i h w) -> p i h w", i=n, h=H, w=W)


def _dedup_ldweights(nc):
    """Remove InstLdweights whose matmul reuses the weights already loaded
    into the same PE region. The PE array retains its stationary operand
    between matmuls, so a matmul with no preceding LDWEIGHTS streams
    against the previously loaded weights (verified on hardware). A
    full-array load invalidates all quadrant entries and vice versa.
    Sync waits/updates on a removed LDWEIGHTS migrate to its matmul."""
    kept = removed = 0
    for f in nc.m.functions:
        for b in f.blocks:
            insts = list(b.instructions)
            last = {}
            dead = []
            i = 0
            while i < len(insts):
                ins = insts[i]
                if isinstance(ins, mybir.InstLdweights):
                    assert i + 1 < len(insts), "trailing LDWEIGHTS"
                    mm = insts[i + 1]
                    assert isinstance(mm, mybir.InstMatmult), (
                        f"LDWEIGHTS not followed by matmul: {type(mm).__name__}")
                    sig = str(mm.ins[1])
                    tp = tuple(mm.tile_position)
                    full = tuple(mm.tile_size) == (128, 128)
                    if last.get((tp, full)) == sig:
                        si = ins.sync_info
                        if si is not None and (len(si.on_wait) or len(si.on_update)):
                            msi = mm.sync_info
                            ow = list(si.on_wait)
                            ou = list(si.on_update)
                            if msi is not None:
                                ow += list(msi.on_wait)
                                ou += list(msi.on_update)
                            mm.sync_info = mybir.SyncInfo(on_wait=ow, on_update=ou)
                        dead.append(ins)
                        removed += 1
                    else:
                        if full:
                            last.clear()
                        else:
                            last.pop((tp, True), None)
                            last.pop(((0, 0), True), None)
                        last[(tp, full)] = sig
                        kept += 1
                    i += 2
                    continue
                assert not isinstance(ins, mybir.InstMatmult), "matmul without LDWEIGHTS"
                i += 1
            for d in dead:
                b.instructions.remove(d)
    assert kept + removed == 1184 and removed >= 600, (kept, removed)


def _build():
    nc = bacc.Bacc("TRN2", target_bir_lowering=False, debug=False,
                   num_devices=NCORES)
    x_d = nc.dram_tensor("x", [128, SLOTS, H, W], F32, kind="ExternalInput")
    w1_d = nc.dram_tensor("w1", [128, 9, 64], BF16, kind="ExternalInput")
    w2_d = nc.dram_tensor("w2", [128, 9, 64], BF16, kind="ExternalInput")
    eye_d = nc.dram_tensor("eye", [128, 128], BF16, kind="ExternalInput")
    cst_d = nc.dram_tensor("cst", [128, 2], F32, kind="ExternalInput")
    msk_d = nc.dram_tensor("msk", [64, 3, H, W], F32, kind="ExternalInput")
    o_d = nc.dram_tensor("o", [128, SLOTS, H, W], BF16, kind="ExternalOutput")

    with tile.TileContext(nc) as tc:
        with (
            tc.tile_pool(name="singles", bufs=1) as singles,
            tc.tile_pool(name="xin", bufs=8) as xin_pool,
            tc.tile_pool(name="outp", bufs=4) as out_pool,
            tc.tile_pool(name="pads", bufs=1) as pad_pool,
            tc.tile_pool(name="ps", bufs=4, space="PSUM") as ps_pool,
        ):
            w1_sb = singles.tile([128, 9, 64], BF16, name="w1_sb")
            w2_sb = singles.tile([128, 9, 64], BF16, name="w2_sb")
            eye_sb = singles.tile([128, 128], BF16, name="eye_sb")
            cst_sb = singles.tile([128, 2], F32, name="cst_sb")
            msk_sb = singles.tile([64, 3, H, W], F32, name="msk_sb")
            m2x_sb = singles.tile([64, H, W], F32, name="m2x_sb")
            warm_sb = singles.tile([128, 1], F32, name="warm_sb")

            # Persistent zero-padded 9x9 tiles: borders are zeroed once in
            # the prologue and never rewritten (compute writes interiors).
            xpads = [pad_pool.tile([128, 16, 9, 9], BF16, name=f"xpad{i}",
                                   tag=f"xpad{i}") for i in range(NXP)]
            y1pads = [pad_pool.tile([128, 16, 9, 9], BF16, name=f"y1pad{i}",
                                    tag=f"y1pad{i}") for i in range(RPS)]

            xin_tiles = {}

            def emit_in_dma(r, q):
                t = xin_pool.tile([128, 16, H, W], F32, name="xin_q")
                q.dma_start(t[:], x_d[:, 16 * r:16 * r + 16])
                xin_tiles[r] = t

            def emit_pad_cast(r):
                nc.vector.tensor_copy(xpads[r % NXP][:, :, 1:8, 1:8],
                                      xin_tiles.pop(r)[:])

            def emit_chain_mms(pads, pr, w_sb, ps_r, t, r, start):
                dh, dw = t // 3, t % 3
                for (j, lane, cg) in CHAINS:
                    rhs = pads[r % pr][64 * lane:64 * lane + 64,
                                       8 * j:8 * j + 8, dh:dh + 7, dw:dw + 7]
                    lhsT = w_sb[64 * lane:64 * lane + 64, t, :]
                    out = ps_r[64 * cg:64 * cg + 64, j, 0:FD]
                    nc.tensor.matmul(out, lhsT, rhs, start=start,
                                     stop=(t == 8))

            def emit_resid_mms(r, ps_r):
                # seed the conv2 accumulation with the identity residual:
                # ps[:, j] = I.T @ x  (full-array matmul, exact in bf16)
                xp = xpads[r % NXP]
                for j in range(2):
                    nc.tensor.matmul(
                        ps_r[:, j, 0:FD], eye_sb[:],
                        xp[:, 8 * j:8 * j + 8, 1:8, 1:8],
                        start=True, stop=False, skip_group_check=True)

            def emit_drain(k, r, ps_r):
                # conv1 bank -> relu(psum + shift1) -> y1pad interior;
                # pair 0 on ACT, pair 1 on DVE so rounds drain in ~600ns
                yp = y1pads[r % RPS]
                nc.scalar.activation(
                    out=yp[:, 0:8, 1:8, 1:8], in_=_psum_view(ps_r, 0),
                    func=mybir.ActivationFunctionType.Relu,
                    bias=cst_sb[:, 0:1], scale=1.0)
                nc.vector.tensor_scalar(
                    yp[:, 8:16, 1:8, 1:8], _psum_view(ps_r, 1),
                    cst_sb[:, 0:1], 0.0,
                    mybir.AluOpType.add, mybir.AluOpType.max)
                if k == 0 and r == 0:
                    # critic mask 1 on relu(bn1(conv1)) of batch element 0
                    tgt = yp[0:64, 0, 1:8, 1:8]
                    nc.vector.tensor_mul(tgt, tgt, msk_sb[:, 0, :, :])

            def emit_finish(k, r, ps_r):
                # conv2 bank already holds conv2 + x; apply mask2 (elem 0)
                # as mask*(conv2+x) + (1-mask)*x, then relu -> DMA out
                if k == 0 and r == 0:
                    tgt = ps_r[0:64, 0, 0:H * W].rearrange(
                        "p (h w) -> p h w", h=H, w=W)
                    nc.vector.tensor_mul(
                        m2x_sb[:], xpads[0][0:64, 0, 1:8, 1:8],
                        msk_sb[:, 2, :, :])
                    nc.vector.tensor_mul(tgt, tgt, msk_sb[:, 1, :, :])
                    nc.vector.tensor_add(tgt, tgt, m2x_sb[:])
                views = [_psum_view(ps_r, j) for j in range(2)]
                out_q = out_pool.tile([128, 16, H, W], BF16, name="out_q")
                nc.vector.tensor_scalar(
                    out_q[:, 0:8], views[0], cst_sb[:, 1:2], 0.0,
                    mybir.AluOpType.add, mybir.AluOpType.max)
                nc.scalar.activation(
                    out=out_q[:, 8:16], in_=views[1],
                    func=mybir.ActivationFunctionType.Relu,
                    bias=cst_sb[:, 1:2], scale=1.0)
                if r == NR - 1:
                    # final round: two half-DMAs so the kernel tail is short
                    nc.sync.dma_start(o_d[:, 16 * r:16 * r + 8], out_q[:, 0:8])
                    nc.sync.dma_start(o_d[:, 16 * r + 8:16 * r + 16],
                                      out_q[:, 8:16])
                else:
                    nc.sync.dma_start(o_d[:, 16 * r:16 * r + 16], out_q[:])

            # ---------------- prologue ----------------
            # DVE: first pad border zero (gates the first conv round)
            nc.vector.memset(xpads[0][:], 0.0)
            # ACT queue: w1 first (needed by the first LDWEIGHTS), then the
            # activation-table prewarm rides behind it
            nc.scalar.dma_start(w1_sb[:], w1_d[:])
            nc.scalar.memzero(warm_sb[:])
            # sync ring: first superblock's x
            for r in range(4):
                emit_in_dma(r, nc.sync)
            # constants on the SWDGE ring
            nc.gpsimd.dma_start(cst_sb[:], cst_d[:])
            nc.gpsimd.dma_start(w2_sb[:], w2_d[:])
            nc.gpsimd.dma_start(eye_sb[:], eye_d[:])
            nc.gpsimd.dma_start(msk_sb[:], msk_d[:])
            # second superblock's x on the ACT ring
            for r in range(4, 8):
                emit_in_dma(r, nc.scalar)
            # remaining pad borders: DVE interleaved with the first casts;
            # the rest off the critical path on gpsimd
            emit_pad_cast(0)
            nc.vector.memset(y1pads[0][:], 0.0)
            nc.vector.memset(xpads[1][:], 0.0)
            emit_pad_cast(1)
            nc.vector.memset(xpads[2][:], 0.0)
            emit_pad_cast(2)
            nc.gpsimd.memset(xpads[3][:], 0.0)
            emit_pad_cast(3)
            nc.gpsimd.memset(y1pads[1][:], 0.0)
            nc.gpsimd.memset(y1pads[2][:], 0.0)
            nc.gpsimd.memset(y1pads[3][:], 0.0)
            for i in range(4, NXP):
                nc.gpsimd.memset(xpads[i][:], 0.0)

            # ---------------- main loop ----------------
            for k in range(NSB):
                rounds = list(range(RPS * k, RPS * k + RPS))
                # conv1 phase (tap-outer over the superblock's rounds)
                ps1 = {r: ps_pool.tile([128, 2, 512], F32, name="ps_t")
                       for r in rounds}
                if k == 0:
                    # round 0 tap-inner first: it only needs the first DMA,
                    # so matmuls start while rounds 1-3 are still in flight
                    for t in range(9):
                        emit_chain_mms(xpads, NXP, w1_sb, ps1[0], t, 0,
                                       t == 0)
                    emit_drain(k, 0, ps1[0])
                    for t in range(9):
                        for r in rounds[1:]:
                            emit_chain_mms(xpads, NXP, w1_sb, ps1[r], t, r,
                                           t == 0)
                            if t == 8:
                                emit_drain(k, r, ps1[r])
                else:
                    for t in range(9):
                        for r in rounds:
                            emit_chain_mms(xpads, NXP, w1_sb, ps1[r], t, r,
                                           t == 0)
                            if t == 8:
                                emit_drain(k, r, ps1[r])

                # prefetch: x DMAs for superblock k+2, pad casts for k+1
                if k + 2 < NSB:
                    for r in range(RPS * (k + 2), RPS * (k + 2) + RPS):
                        emit_in_dma(r, nc.sync)
                if k + 1 < NSB:
                    for r in range(RPS * (k + 1), RPS * (k + 1) + RPS):
                        emit_pad_cast(r)

                # conv2 phase: residual seeds first, then the taps
                ps2 = {r: ps_pool.tile([128, 2, 512], F32, name="ps_t")
                       for r in rounds}
                for r in rounds:
                    emit_resid_mms(r, ps2[r])
                for t in range(9):
                    for r in rounds:
                        emit_chain_mms(y1pads, RPS, w2_sb, ps2[r], t, r,
                                       False)
                        if t == 8:
                            emit_finish(k, r, ps2[r])

    _dedup_ldweights(nc)
    nc.compile()
    return nc


def _get_nc():
    if "nc" not in _CACHE:
        _CACHE["nc"] = _build()
    return _CACHE["nc"]


# slot mapping: slot s (0..255) within a lane h decomposes as
#   k = s//64 (superblock), rr = (s//16)%4 (round), j = (s//8)%2 (pair),
#   i = s%8  ->  image index = 128k + 32rr + 16j + 8h + i
_S = np.arange(SLOTS)
_IMG = (128 * (_S // 64) + 32 * ((_S // 16) % 4) + 16 * ((_S // 8) % 2)
        + (_S % 8))
_IMG = np.stack([_IMG, _IMG + 8])          # [2 lanes, 256 slots]
_I = np.arange(BPC)
_INV_H = (_I % 16) // 8
_INV_S = 64 * (_I // 128) + 16 * ((_I % 128) // 32) + 8 * ((_I % 32) // 16) \
    + (_I % 8)


def _host_pack(x, w1, g1, b1, m1, v1, w2, g2, b2, m2, v2, mask1, mask2):
    x = np.asarray(x, np.float32)
    scale1 = np.asarray(g1, np.float32) / np.sqrt(np.asarray(v1, np.float32) + EPS)
    shift1 = np.asarray(b1, np.float32) - np.asarray(m1, np.float32) * scale1
    scale2 = np.asarray(g2, np.float32) / np.sqrt(np.asarray(v2, np.float32) + EPS)
    shift2 = np.asarray(b2, np.float32) - np.asarray(m2, np.float32) * scale2

    def pack_w(w, scale):
        ws = np.asarray(w, np.float32) * scale[:, None, None, None]
        # [co, ci, kh, kw] -> [ci, tap, co], duplicated into both lanes
        lhsT = ws.transpose(1, 2, 3, 0).reshape(64, 9, 64)
        return np.ascontiguousarray(np.tile(lhsT, (2, 1, 1)).astype(NP_BF16))

    wdev1, wdev2 = pack_w(w1, scale1), pack_w(w2, scale2)
    eye = np.ascontiguousarray(np.eye(128, dtype=NP_BF16))
    cst = np.tile(np.stack([shift1, shift2], 1), (2, 1))
    cst = np.ascontiguousarray(cst.astype(np.float32))

    x_cores = x.reshape(NCORES, BPC, C, H, W)
    # [core, lane, slot, ch, h, w] -> [core, lane*ch, slot, h, w]
    xdev = np.ascontiguousarray(
        x_cores[:, _IMG].transpose(0, 1, 3, 2, 4, 5).reshape(
            NCORES, 128, SLOTS, H, W))

    m1_ = np.asarray(mask1, np.float32)
    m2_ = np.asarray(mask2, np.float32)
    msk0 = np.ascontiguousarray(np.stack([m1_, m2_, 1.0 - m2_], 1))
    mskn = np.ascontiguousarray(np.stack(
        [np.ones_like(m1_), np.ones_like(m2_), np.zeros_like(m2_)], 1))

    in_maps = []
    for c in range(NCORES):
        in_maps.append({
            "x": xdev[c],
            "w1": wdev1,
            "w2": wdev2,
            "eye": eye,
            "cst": cst,
            "msk": msk0 if c == 0 else mskn,
        })
    return in_maps


def _host_unpack(results):
    o = np.stack([results[c]["o"] for c in range(NCORES)])
    o = np.asarray(o, np.float32).reshape(NCORES, 2, C, SLOTS, H, W)
    o = o.transpose(0, 1, 3, 2, 4, 5)      # [core, lane, slot, ch, h, w]
    out = o[:, _INV_H, _INV_S]             # [core, img, ch, h, w]
    return np.ascontiguousarray(out.reshape(B, C, H, W))


def run(trace=False, **inputs):
    nc = _get_nc()
    in_maps = _host_pack(**inputs)
    res = run_bass_kernel_spmd(nc, in_maps, core_ids=list(range(NCORES)),
                               trace=trace)
    return _host_unpack(res.results), res


def kernel(**inputs) -> np.ndarray:
    out, _ = run(trace=False, **inputs)
    return out


# revision 9
# speedup vs baseline: 1.1648x; 1.0918x over previous
"""Trainium2 Bass kernel: ResNet BasicBlock (conv3x3-BN-ReLU-mask-conv3x3-mask-BN-residual-ReLU).

Problem shape: x[4096, 64, 7, 7], both convs 64->64 3x3 pad 1.

Strategy (pure data parallel, 8 cores, 512 images/core):
  * Channels live on SBUF partitions. Two 64-channel image streams are
    stacked into the 128 partitions ("lane0" -> partitions 0-63,
    "lane1" -> 64-127) so elementwise engines run at full width.
  * A 3x3 conv is 9 shifted 64x64 matmuls accumulated in PSUM. Images are
    zero-padded to 9x9 on-chip; each tap reads a strided window of the
    padded tile. Matmul operands are bf16; accumulation is fp32 in PSUM.
  * The 128x128 PE array is split into 4 64x64 quadrants via the matmul
    base partitions. Four independent chains (2 pairs x 2 lanes) run
    concurrently, fully utilizing the array despite C=64.
  * Tap-OUTER loop over superblocks of 4 rounds: all rounds of a tap run
    back-to-back with the same stationary weights, and redundant
    LDWEIGHTS are deleted from the BIR post-schedule (the PE reuses
    loaded weights), eliminating the per-matmul weight-load tax.
  * The identity residual is a full-array identity matmul that seeds each
    conv2 PSUM accumulation group with x (bf16 identity is exact), so no
    vector add sits on the bank-release critical path.
  * Every producer/consumer pair-split resource (PSUM banks, y1 pads,
    output staging) uses separate tiles per pair so the ACT and DVE
    engines never serialize on coarse same-tile dependencies.
  * BN scales fold into conv weights on the host; BN shifts are
    per-partition bias operands of the drain/final activations.
  * Output returns as bf16; x arrives fp32 (the residual uses the bf16
    conv input, costing <=2^-9 relative - within the error budget).
  * Critic masks only touch batch element 0; cores 1-7 get neutral
    masks. Since the residual is pre-added, mask2 is applied as
    mask*(conv2+x) + (1-mask)*x on the first image's region.
  * Prologue: DVE cast warmup (the first CAST otherwise pays a ~8us
    one-time cost) and ~3.5us of dummy matmuls to lift the PE HAM clock
    gate to 2.4 GHz before the real stream begins.
"""

import ml_dtypes
import numpy as np

import concourse.bass as bass  # noqa: F401  (engine namespaces live on the nc object)
import concourse.tile as tile
from concourse import bacc, mybir
from concourse.bass_utils import run_bass_kernel_spmd

F32 = mybir.dt.float32
BF16 = mybir.dt.bfloat16
NP_BF16 = ml_dtypes.bfloat16
EPS = 1e-5
B, C, H, W = 4096, 64, 7, 7
NCORES = 8
BPC = B // NCORES          # 512 images per core
SLOTS = BPC // 2           # 256 image slots per lane
N = 8                      # images per chain visit
NR = 16                    # rounds (each = 4 chains x 8 images = 32 images)
NSB = 4                    # superblocks of 4 rounds
RPS = 4                    # rounds per superblock
NXP = 8                    # xpad ring depth (2 superblocks)
FD = N * H * W             # 392 psum columns per chain
NWARM = 36                 # HAM warmup matmuls

# (pair_in_round, lane, colgroup): the 4 concurrent chains of a round.
CHAINS = [(0, 0, 0), (1, 1, 0), (0, 1, 1), (1, 0, 1)]

_CACHE = {}


def _pv(ps, n=N):
    """[128, n, 7, 7] view of a [128, 512] single-bank psum tile."""
    return ps[:, 0:n * H * W].rearrange("p (i h w) -> p i h w",
                                        i=n, h=H, w=W)


def _dedup_ldweights(nc):
    """Remove InstLdweights whose matmul reuses the weights already loaded
    into the same PE region. The PE array retains its stationary operand
    between matmuls, so a matmul with no preceding LDWEIGHTS streams
    against the previously loaded weights (verified on hardware). A
    full-array load invalidates all quadrant entries and vice versa.
    Sync waits/updates on a removed LDWEIGHTS migrate to its matmul."""
    kept = removed = 0
    for f in nc.m.functions:
        for b in f.blocks:
            insts = list(b.instructions)
            last = {}
            dead = []
            i = 0
            while i < len(insts):
                ins = insts[i]
                if isinstance(ins, mybir.InstLdweights):
                    assert i + 1 < len(insts), "trailing LDWEIGHTS"
                    mm = insts[i + 1]
                    assert isinstance(mm, mybir.InstMatmult), (
                        f"LDWEIGHTS not followed by matmul: {type(mm).__name__}")
                    sig = str(mm.ins[1])
                    tp = tuple(mm.tile_position)
                    full = tuple(mm.tile_size) == (128, 128)
                    if last.get((tp, full)) == sig:
                        si = ins.sync_info
                        if si is not None and (len(si.on_wait) or len(si.on_update)):
                            msi = mm.sync_info
                            ow = list(si.on_wait)
                            ou = list(si.on_update)
                            if msi is not None:
                                ow += list(msi.on_wait)
                                ou += list(msi.on_update)
                            mm.sync_info = mybir.SyncInfo(on_wait=ow, on_update=ou)
                        dead.append(ins)
                        removed += 1
                    else:
                        if full:
                            last.clear()
                        else:
                            last.pop((tp, True), None)
                            last.pop(((0, 0), True), None)
                        last[(tp, full)] = sig
                        kept += 1
                    i += 2
                    continue
                assert not isinstance(ins, mybir.InstMatmult), "matmul without LDWEIGHTS"
                i += 1
            for d in dead:
                b.instructions.remove(d)
    assert kept + removed == 1184 + NWARM and removed >= 600, (kept, removed)


def _build():
    nc = bacc.Bacc("TRN2", target_bir_lowering=False, debug=False,
                   num_devices=NCORES)
    x_d = nc.dram_tensor("x", [128, SLOTS, H, W], F32, kind="ExternalInput")
    w1_d = nc.dram_tensor("w1", [128, 9, 64], BF16, kind="ExternalInput")
    w2_d = nc.dram_tensor("w2", [128, 9, 64], BF16, kind="ExternalInput")
    eye_d = nc.dram_tensor("eye", [128, 128], BF16, kind="ExternalInput")
    cst_d = nc.dram_tensor("cst", [128, 2], F32, kind="ExternalInput")
    msk_d = nc.dram_tensor("msk", [64, 3, H, W], F32, kind="ExternalInput")
    o_d = nc.dram_tensor("o", [128, SLOTS, H, W], BF16, kind="ExternalOutput")

    with tile.TileContext(nc) as tc:
        with (
            tc.tile_pool(name="singles", bufs=1) as singles,
            tc.tile_pool(name="xin", bufs=8) as xin_pool,
            tc.tile_pool(name="outa", bufs=4) as outa_pool,
            tc.tile_pool(name="outb", bufs=4) as outb_pool,
            tc.tile_pool(name="pads", bufs=1) as pad_pool,
            tc.tile_pool(name="ps", bufs=8, space="PSUM") as ps_pool,
        ):
            w1_sb = singles.tile([128, 9, 64], BF16, name="w1_sb")
            w2_sb = singles.tile([128, 9, 64], BF16, name="w2_sb")
            eye_sb = singles.tile([128, 128], BF16, name="eye_sb")
            cst_sb = singles.tile([128, 2], F32, name="cst_sb")
            msk_sb = singles.tile([64, 3, H, W], F32, name="msk_sb")
            m2x_sb = singles.tile([64, H, W], F32, name="m2x_sb")
            warm_sb = singles.tile([128, 1], F32, name="warm_sb")
            wrm_src = singles.tile([128, 64], F32, name="wrm_src")
            wrm_bf = singles.tile([128, 64], BF16, name="wrm_bf")

            # Persistent zero-padded 9x9 tiles: borders are zeroed once in
            # the prologue and never rewritten (compute writes interiors).
            xpads = [pad_pool.tile([128, 16, 9, 9], BF16, name=f"xpad{i}",
                                   tag=f"xpad{i}") for i in range(NXP)]
            # conv1 outputs, one tile per (round, pair) so the two drain
            # engines never serialize on a shared tile
            y1p = [[pad_pool.tile([128, 8, 9, 9], BF16, name=f"y1p{i}_{j}",
                                  tag=f"y1p{i}_{j}") for j in range(2)]
                   for i in range(RPS)]

            xin_tiles = {}

            def emit_in_dma(r, q):
                t = xin_pool.tile([128, 16, H, W], F32, name="xin_q")
                q.dma_start(t[:], x_d[:, 16 * r:16 * r + 16])
                xin_tiles[r] = t

            def emit_pad_cast(r):
                nc.vector.tensor_copy(xpads[r % NXP][:, :, 1:8, 1:8],
                                      xin_tiles.pop(r)[:])

            def emit_chain_mms(srcs, w_sb, ps_pair, t, start):
                # srcs: per-pair (tile, slot_base); ps_pair: per-pair bank
                dh, dw = t // 3, t % 3
                for (j, lane, cg) in CHAINS:
                    pad, base = srcs[j]
                    rhs = pad[64 * lane:64 * lane + 64,
                              base:base + 8, dh:dh + 7, dw:dw + 7]
                    lhsT = w_sb[64 * lane:64 * lane + 64, t, :]
                    out = ps_pair[j][64 * cg:64 * cg + 64, 0:FD]
                    nc.tensor.matmul(out, lhsT, rhs, start=start,
                                     stop=(t == 8))

            def emit_resid_mms(r, ps_pair):
                # seed the conv2 accumulation with the identity residual:
                # ps_j = I.T @ x  (full-array matmul, exact in bf16)
                xp = xpads[r % NXP]
                for j in range(2):
                    nc.tensor.matmul(
                        ps_pair[j][:, 0:FD], eye_sb[:],
                        xp[:, 8 * j:8 * j + 8, 1:8, 1:8],
                        start=True, stop=False, skip_group_check=True)

            def emit_drain(k, r, ps_pair):
                # conv1 banks -> relu(psum + shift1) -> y1 pad interiors;
                # pair 0 on ACT, pair 1 on DVE, fully independent tiles
                ya, yb = y1p[r % RPS]
                nc.scalar.activation(
                    out=ya[:, :, 1:8, 1:8], in_=_pv(ps_pair[0]),
                    func=mybir.ActivationFunctionType.Relu,
                    bias=cst_sb[:, 0:1], scale=1.0)
                nc.vector.tensor_scalar(
                    yb[:, :, 1:8, 1:8], _pv(ps_pair[1]),
                    cst_sb[:, 0:1], 0.0,
                    mybir.AluOpType.add, mybir.AluOpType.max)
                if k == 0 and r == 0:
                    # critic mask 1 on relu(bn1(conv1)) of batch element 0
                    tgt = ya[0:64, 0, 1:8, 1:8]
                    nc.vector.tensor_mul(tgt, tgt, msk_sb[:, 0, :, :])

            def emit_finish(k, r, ps_pair):
                # conv2 banks already hold conv2 + x; apply mask2 (elem 0)
                # as mask*(conv2+x) + (1-mask)*x, then relu -> DMA out.
                # pair 1 final on ACT first (independent bank), pair 0 on DVE
                out_b = outb_pool.tile([128, 8, H, W], BF16, name="out_b")
                nc.scalar.activation(
                    out=out_b[:], in_=_pv(ps_pair[1]),
                    func=mybir.ActivationFunctionType.Relu,
                    bias=cst_sb[:, 1:2], scale=1.0)
                if k == 0 and r == 0:
                    tgt = ps_pair[0][0:64, 0:H * W].rearrange(
                        "p (h w) -> p h w", h=H, w=W)
                    nc.vector.tensor_mul(
                        m2x_sb[:], xpads[0][0:64, 0, 1:8, 1:8],
                        msk_sb[:, 2, :, :])
                    nc.vector.tensor_mul(tgt, tgt, msk_sb[:, 1, :, :])
                    nc.vector.tensor_add(tgt, tgt, m2x_sb[:])
                out_a = outa_pool.tile([128, 8, H, W], BF16, name="out_a")
                nc.vector.tensor_scalar(
                    out_a[:], _pv(ps_pair[0]), cst_sb[:, 1:2], 0.0,
                    mybir.AluOpType.add, mybir.AluOpType.max)
                nc.sync.dma_start(o_d[:, 16 * r + 8:16 * r + 16], out_b[:])
                nc.sync.dma_start(o_d[:, 16 * r:16 * r + 8], out_a[:])

            # ---------------- prologue ----------------
            # DVE: warmups (the first CAST pays a large one-time cost if it
            # lands cold), then the first pad border zero
            nc.vector.memset(wrm_src[:], 0.0)
            nc.vector.tensor_copy(wrm_bf[:], wrm_src[:])
            nc.vector.memset(xpads[0][:], 0.0)
            # sync ring: w1 first (needed by the first LDWEIGHTS), then the
            # first superblock's x
            nc.sync.dma_start(w1_sb[:], w1_d[:])
            for r in range(4):
                emit_in_dma(r, nc.sync)
            # ACT-table prewarm rides on the scalar queue
            nc.scalar.memzero(warm_sb[:])
            # constants on the SWDGE ring
            nc.gpsimd.dma_start(cst_sb[:], cst_d[:])
            nc.gpsimd.dma_start(w2_sb[:], w2_d[:])
            nc.gpsimd.dma_start(eye_sb[:], eye_d[:])
            nc.gpsimd.dma_start(msk_sb[:], msk_d[:])
            # second superblock's x on the ACT ring
            for r in range(4, 8):
                emit_in_dma(r, nc.scalar)
            # HAM warmup: dummy matmuls on scratch data keep the PE busy
            # through the DMA wait so the real stream starts at 2.4 GHz
            ps1 = {0: [ps_pool.tile([128, 512], F32, name="ps_t")
                       for _ in range(2)]}
            for i in range(NWARM):
                nc.tensor.matmul(ps1[0][i % 2][0:64, 0:64],
                                 wrm_bf[0:64, :], wrm_bf[0:64, :],
                                 start=True, stop=True,
                                 skip_group_check=True)
            # remaining pad borders: DVE interleaved with the first casts;
            # the rest off the critical path on gpsimd
            emit_pad_cast(0)
            nc.vector.memset(y1p[0][1][:], 0.0)
            nc.vector.memset(xpads[1][:], 0.0)
            emit_pad_cast(1)
            nc.vector.memset(y1p[0][0][:], 0.0)
            nc.vector.memset(xpads[2][:], 0.0)
            emit_pad_cast(2)
            nc.gpsimd.memset(xpads[3][:], 0.0)
            emit_pad_cast(3)
            for i in range(1, RPS):
                nc.gpsimd.memset(y1p[i][0][:], 0.0)
                nc.gpsimd.memset(y1p[i][1][:], 0.0)
            for i in range(4, NXP):
                nc.gpsimd.memset(xpads[i][:], 0.0)

            # ---------------- main loop ----------------
            for k in range(NSB):
                rounds = list(range(RPS * k, RPS * k + RPS))
                # conv1 phase (tap-outer over the superblock's rounds)
                for r in rounds:
                    if (k, r) != (0, 0):
                        ps1[r] = [ps_pool.tile([128, 512], F32, name="ps_t")
                                  for _ in range(2)]

                def xsrc(r):
                    xp = xpads[r % NXP]
                    return [(xp, 0), (xp, 8)]

                if k == 0:
                    # round 0 tap-inner first: it only needs the first DMA,
                    # so matmuls start while rounds 1-3 are still in flight
                    for t in range(9):
                        emit_chain_mms(xsrc(0), w1_sb, ps1[0], t, t == 0)
                    emit_drain(k, 0, ps1[0])
                    for t in range(9):
                        for r in rounds[1:]:
                            emit_chain_mms(xsrc(r), w1_sb, ps1[r], t, t == 0)
                            if t == 8:
                                emit_drain(k, r, ps1[r])
                else:
                    for t in range(9):
                        for r in rounds:
                            emit_chain_mms(xsrc(r), w1_sb, ps1[r], t, t == 0)
                            if t == 8:
                                emit_drain(k, r, ps1[r])

                # prefetch: x DMAs for superblock k+2, pad casts for k+1
                if k + 2 < NSB:
                    for r in range(RPS * (k + 2), RPS * (k + 2) + RPS):
                        emit_in_dma(r, nc.sync)
                if k + 1 < NSB:
                    for r in range(RPS * (k + 1), RPS * (k + 1) + RPS):
                        emit_pad_cast(r)

                # conv2 phase: residual seeds first, then the taps
                ps2 = {r: [ps_pool.tile([128, 512], F32, name="ps_t")
                           for _ in range(2)] for r in rounds}
                for r in rounds:
                    emit_resid_mms(r, ps2[r])
                for t in range(9):
                    for r in rounds:
                        y2src = [(y1p[r % RPS][0], 0), (y1p[r % RPS][1], 0)]
                        emit_chain_mms(y2src, w2_sb, ps2[r], t, False)
                        if t == 8:
                            emit_finish(k, r, ps2[r])

    _dedup_ldweights(nc)
    nc.compile()
    return nc


def _get_nc():
    if "nc" not in _CACHE:
        _CACHE["nc"] = _build()
    return _CACHE["nc"]


# slot mapping: slot s (0..255) within a lane h decomposes as
#   k = s//64 (superblock), rr = (s//16)%4 (round), j = (s//8)%2 (pair),
#   i = s%8  ->  image index = 128k + 32rr + 16j + 8h + i
_S = np.arange(SLOTS)
_IMG = (128 * (_S // 64) + 32 * ((_S // 16) % 4) + 16 * ((_S // 8) % 2)
        + (_S % 8))
_IMG = np.stack([_IMG, _IMG + 8])          # [2 lanes, 256 slots]
_I = np.arange(BPC)
_INV_H = (_I % 16) // 8
_INV_S = 64 * (_I // 128) + 16 * ((_I % 128) // 32) + 8 * ((_I % 32) // 16) \
    + (_I % 8)


def _host_pack(x, w1, g1, b1, m1, v1, w2, g2, b2, m2, v2, mask1, mask2):
    x = np.asarray(x, np.float32)
    scale1 = np.asarray(g1, np.float32) / np.sqrt(np.asarray(v1, np.float32) + EPS)
    shift1 = np.asarray(b1, np.float32) - np.asarray(m1, np.float32) * scale1
    scale2 = np.asarray(g2, np.float32) / np.sqrt(np.asarray(v2, np.float32) + EPS)
    shift2 = np.asarray(b2, np.float32) - np.asarray(m2, np.float32) * scale2

    def pack_w(w, scale):
        ws = np.asarray(w, np.float32) * scale[:, None, None, None]
        # [co, ci, kh, kw] -> [ci, tap, co], duplicated into both lanes
        lhsT = ws.transpose(1, 2, 3, 0).reshape(64, 9, 64)
        return np.ascontiguousarray(np.tile(lhsT, (2, 1, 1)).astype(NP_BF16))

    wdev1, wdev2 = pack_w(w1, scale1), pack_w(w2, scale2)
    eye = np.ascontiguousarray(np.eye(128, dtype=NP_BF16))
    cst = np.tile(np.stack([shift1, shift2], 1), (2, 1))
    cst = np.ascontiguousarray(cst.astype(np.float32))

    x_cores = x.reshape(NCORES, BPC, C, H, W)
    # [core, lane, slot, ch, h, w] -> [core, lane*ch, slot, h, w]
    xdev = np.ascontiguousarray(
        x_cores[:, _IMG].transpose(0, 1, 3, 2, 4, 5).reshape(
            NCORES, 128, SLOTS, H, W))

    m1_ = np.asarray(mask1, np.float32)
    m2_ = np.asarray(mask2, np.float32)
    msk0 = np.ascontiguousarray(np.stack([m1_, m2_, 1.0 - m2_], 1))
    mskn = np.ascontiguousarray(np.stack(
        [np.ones_like(m1_), np.ones_like(m2_), np.zeros_like(m2_)], 1))

    in_maps = []
    for c in range(NCORES):
        in_maps.append({
            "x": xdev[c],
            "w1": wdev1,
            "w2": wdev2,
            "eye": eye,
            "cst": cst,
            "msk": msk0 if c == 0 else mskn,
        })
    return in_maps


def _host_unpack(results):
    o = np.stack([results[c]["o"] for c in range(NCORES)])
    o = np.asarray(o, np.float32).reshape(NCORES, 2, C, SLOTS, H, W)
    o = o.transpose(0, 1, 3, 2, 4, 5)      # [core, lane, slot, ch, h, w]
    out = o[:, _INV_H, _INV_S]             # [core, img, ch, h, w]
    return np.ascontiguousarray(out.reshape(B, C, H, W))


def run(trace=False, **inputs):
    nc = _get_nc()
    in_maps = _host_pack(**inputs)
    res = run_bass_kernel_spmd(nc, in_maps, core_ids=list(range(NCORES)),
                               trace=trace)
    return _host_unpack(res.results), res


def kernel(**inputs) -> np.ndarray:
    out, _ = run(trace=False, **inputs)
    return out
